# revision 50
# baseline (speedup 1.0000x reference)
"""Trainium2 Bass kernel for nn_CNN_88287347736632 (dense_cnn).

kernel(**inputs) takes the FULL unsharded inputs (as produced by
reference.setup_inputs) and returns the FULL [16, 108, 25] float32 output.

Sharding: pure data parallel over 8 NeuronCores — batch rows 2k, 2k+1 go to
core k. All conv/BN parameters are replicated (BN is folded into conv
weights/bias on the host).

Per-core mapping:
  - 216 sliding windows ([1,144,15] images), processed in waves of 16
    (one partial 12-window wave per batch row: 108 = 6*16 + 12).
  - Convs are tensor-engine matmuls; taps (dh,dw) are accumulating PSUM
    passes reading AP-shifted views of padded SBUF activations;
    tile_position packing runs up to 16 32x32 PE tiles (16 windows)
    concurrently.
  - conv7 (12x9 kernel, K=64*12*9) uses a 2x h-replicated layout giving 54
    full-K=128 passes at M=128.
  - All matmul operands bf16 (fp32 PSUM accumulation); end-to-end rel err
    vs the fp32 reference is ~1.7e-3.
  - mean-pool and the 1x1 conv8 commute: the evacuation of conv7 output
    computes spatial sums via accum_out, and conv8 is a single K=128
    matmul over all 216 window-sums.

Engine balance (the scalar/ACT engine alone cannot keep up with the PE):
  - conv1 (0.4% of FLOPs) runs on the host; its padded activations are
    DMA'd straight into the conv2 input tiles, split across the 3
    DMA-capable engines' queues (one ~22.5GB/s queue per issuing engine).
  - PSUM evacuation (bias+ReLU) is split 3:1 per chunk between the scalar
    engine and the vector engine (tensor_scalar add+max), keeping the
    per-chunk evacuation under the PE's full-clock chunk fill so the PE
    streams without p-state-resetting bubbles.
  - Deep-layer work (conv5/6/7 matmul groups) is emitted at the conv2..4
    layer boundaries of the NEXT wave (inputs >= 1 wave old), so the PE
    FIFO never queues behind a fresh cross-engine dependency.
"""

import os

import numpy as np
import ml_dtypes

import concourse.bass as bass
import concourse.mybir as mybir
import concourse.tile as tile
from concourse import bacc
from concourse.bass_utils import run_bass_kernel_spmd

# conv7 in fp8(e4m3) with DoubleRow perf mode (2 contraction rows per PE
# cell -> ~1.4x tensor throughput on the largest conv). Weights are scaled
# by W7_SCALE before quantization (folded W7 values sit in e4m3's subnormal
# range) and the PSUM is rescaled by 1/W7_SCALE during evacuation.
# DISABLED by default: e4m3's 3-bit mantissa on both operands of the
# biggest contraction yields 2.3e-2 end-to-end rel err, just over the 2e-2
# correctness gate (verified on HW and in host emulation).
FP8_CONV7 = os.environ.get("K_FP8", "0") == "1"
W7_SCALE = 64.0

BF16 = mybir.dt.bfloat16
FP8 = mybir.dt.float8e4
F32 = mybir.dt.float32
RELU = mybir.ActivationFunctionType.Relu
IDENT = mybir.ActivationFunctionType.Identity

EPS = 1e-5
CTX = 7
F = 144
WIN = 15
T_PER_B = 108
N_CORES = 8

CHUNKS_144 = [(0, 29), (29, 29), (58, 29), (87, 29), (116, 28)]
CHUNKS_70 = [(0, 35), (35, 35)]
CHUNKS_68 = [(0, 34), (34, 34)]


# ----------------------------------------------------------------- host prep

def fold_bn(inputs):
    Ws, bs = {}, {}
    for i in range(1, 8):
        W = np.asarray(inputs[f'W{i}'], np.float32)
        b = np.asarray(inputs[f'b{i}'], np.float32)
        g = np.asarray(inputs[f'g{i}'], np.float32)
        be = np.asarray(inputs[f'be{i}'], np.float32)
        m = np.asarray(inputs[f'm{i}'], np.float32)
        v = np.asarray(inputs[f'v{i}'], np.float32)
        s = g / np.sqrt(v + EPS)
        Ws[i] = W * s[:, None, None, None]
        bs[i] = (b - m) * s + be
    return Ws, bs


def wave_plan(T=T_PER_B):
    plan = []
    t0 = 0
    while T - t0 > 12:
        plan.append((t0, 4))
        t0 += 16
    assert T - t0 in (12, 8, 4)
    plan.append((t0, (T - t0) // 4))
    return plan


def host_conv1(xb, W1, b1, plan):
    """conv1 (+folded BN+ReLU) on the host for one batch row, assembled into
    the padded [146,17] per-window layout the device DMAs straight into the
    conv2 input tiles.

    Sliding windows share interior columns with the full-sequence conv; only
    window columns 0 and 14 differ (window zero-padding drops one tap), so we
    compute one full conv + two 2-tap edge convs.
    Returns [n_waves, 4(i), 128(32*slot+ch), 146, 17] bf16.
    """
    T = xb.shape[0]
    U = T + 2 * CTX
    xp = np.pad(xb.T, ((1, 1), (CTX + 1, CTX + 1)))   # [F+2, U+2]
    S = np.lib.stride_tricks.sliding_window_view(xp, (3, 3))  # [F, U, 3, 3]
    W = W1[:, 0]                                       # [32, 3, 3]
    # full conv at shift u; window col j (1..13) of window t = Cf[:, :, t+j]
    Cf = np.tensordot(W, S, axes=([1, 2], [2, 3]))     # [32, F, U]
    # window col 0 drops the dw=0 tap; col 14 drops the dw=2 tap
    C0 = np.tensordot(W[:, :, 1:3], S[:, 0:T, :, 1:3], axes=([1, 2], [2, 3]))
    C14 = np.tensordot(W[:, :, 0:2], S[:, 14:14 + T, :, 0:2], axes=([1, 2], [2, 3]))
    Cf = np.maximum(Cf + b1[:, None, None], 0.0)
    C0 = np.maximum(C0 + b1[:, None, None], 0.0)
    C14 = np.maximum(C14 + b1[:, None, None], 0.0)
    out = np.zeros((len(plan), 4, 128, 146, 17), ml_dtypes.bfloat16)
    for wi, (t0, ncols) in enumerate(plan):
        for s in range(ncols):
            for i in range(4):
                t = t0 + 4 * s + i
                blk = np.empty((32, F, WIN), np.float32)
                blk[:, :, 0] = C0[:, :, t]
                blk[:, :, 1:14] = Cf[:, :, t + 1:t + 14]
                blk[:, :, 14] = C14[:, :, t]
                out[wi, i, 32 * s:32 * s + 32, 1:145, 1:16] = blk
    return out


def prep_weights(Ws, bs, W8, b8):
    d = {}
    for l in (2, 3, 4):
        w = np.zeros((128, 9 * 32), np.float32)
        for g in range(4):
            for k in range(9):
                dh, dw = k // 3, k % 3
                w[32 * g:32 * g + 32, 32 * k:32 * k + 32] = Ws[l][:, :, dh, dw].T
        d[f'w{l}'] = w.astype(ml_dtypes.bfloat16)
    w5 = np.zeros((128, 9 * 64), np.float32)
    for g in range(4):
        for k in range(9):
            dh, dw = k // 3, k % 3
            w5[32 * g:32 * g + 32, 64 * k:64 * k + 64] = Ws[5][:, :, dh, dw].T
    d['w5'] = w5.astype(ml_dtypes.bfloat16)
    w6 = np.zeros((128, 9 * 64), np.float32)
    for p in range(2):
        for k in range(9):
            dh, dw = k // 3, k % 3
            w6[64 * p:64 * p + 64, 64 * k:64 * k + 64] = Ws[6][:, :, dh, dw].T
    d['w6'] = w6.astype(ml_dtypes.bfloat16)
    if FP8_CONV7:
        w7 = np.zeros((64, 54, 2, 128), np.float32)
        for jj in range(6):
            for dw in range(9):
                idx = jj * 9 + dw
                for p in range(2):
                    w7[:, idx, p, :] = Ws[7][:, :, 2 * jj + p, dw].T * W7_SCALE
        d['w7'] = w7.reshape(64, -1).astype(ml_dtypes.float8_e4m3fn)
    else:
        w7 = np.zeros((128, 54 * 128), np.float32)
        for jj in range(6):
            for dw in range(9):
                idx = jj * 9 + dw
                for p in range(2):
                    w7[64 * p:64 * p + 64, 128 * idx:128 * idx + 128] = \
                        Ws[7][:, :, 2 * jj + p, dw].T
        d['w7'] = w7.astype(ml_dtypes.bfloat16)
    d['w8'] = (np.asarray(W8, np.float32)[:, :, 0, 0].T / 69.0).astype(np.float32)
    for l, c in ((2, 32), (3, 32), (4, 32), (5, 64), (6, 64)):
        t = np.zeros((128, 1), np.float32)
        t[:, 0] = np.tile(bs[l], 128 // c)
        d[f'b{l}'] = t
    d['b7'] = bs[7].reshape(128, 1).astype(np.float32)
    d['b8'] = np.asarray(b8, np.float32).reshape(25, 1)
    return d


def host_prepare(inputs, n_cores=N_CORES):
    Ws, bs = fold_bn(inputs)
    wd = prep_weights(Ws, bs, inputs['W8'], inputs['b8'])
    x = np.asarray(inputs['x'], np.float32)
    B = x.shape[0]
    b_per_core = B // n_cores
    plan = wave_plan(x.shape[1])
    in_maps = []
    for c in range(n_cores):
        a1s = [host_conv1(x[c * b_per_core + i], Ws[1], bs[1], plan)
               for i in range(b_per_core)]
        m = dict(wd)
        m['a1'] = np.concatenate([a.reshape(-1) for a in a1s])
        in_maps.append(m)
    return in_maps, plan


# ------------------------------------------------------------- device builder

class ActTile:
    def __init__(self, ap, gsize, windows):
        self.ap = ap
        self.gsize = gsize
        self.windows = windows


def emit(tc, ins, y_ap, n_b=2, T=T_PER_B, repeat=1):
    nc = tc.nc
    ADD = mybir.AluOpType.add
    MAX = mybir.AluOpType.max
    _ctr = [0]

    def nm(base):
        _ctr[0] += 1
        return f"{base}{_ctr[0]}"
    plan = wave_plan(T)
    nwin_total = n_b * T

    import contextlib
    from collections import deque
    stack = contextlib.ExitStack()
    persist = stack.enter_context(tc.tile_pool(name="persist", bufs=1))
    a5p_pool = stack.enter_context(tc.tile_pool(name="a5p", bufs=6))
    a5i_pool = stack.enter_context(tc.tile_pool(name="a5i", bufs=14))
    a6_pool = stack.enter_context(tc.tile_pool(name="a6", bufs=16))
    a7_pool = stack.enter_context(tc.tile_pool(name="a7", bufs=10))
    rep_pool = stack.enter_context(tc.tile_pool(name="rep7", bufs=3))
    psA = stack.enter_context(tc.tile_pool(name="psA", bufs=6, space="PSUM"))
    psB = stack.enter_context(tc.tile_pool(name="psB", bufs=2, space="PSUM"))

    def evac(dst, src, bias_ap, eng):
        # dst = relu(src + bias): ACT activation or DVE/GPSIMD tensor_scalar
        if eng == 'act':
            nc.scalar.activation(dst, src, RELU, bias=bias_ap)
        elif eng == 'dve':
            nc.vector.tensor_scalar(dst, src, bias_ap, 0.0, ADD, MAX)
        else:
            nc.gpsimd.tensor_scalar(dst, src, bias_ap, 0.0, ADD, MAX)

    wt = {}
    w7_spec = ([64, 54 * 256], FP8) if FP8_CONV7 else ([128, 54 * 128], BF16)
    for name, shape, dt in (
        ('w2', [128, 288], BF16), ('w3', [128, 288], BF16),
        ('w4', [128, 288], BF16), ('w5', [128, 576], BF16), ('w6', [128, 576], BF16),
        ('w7', *w7_spec), ('w8', [128, 25], F32),
        ('b2', [128, 1], F32), ('b3', [128, 1], F32),
        ('b4', [128, 1], F32), ('b5', [128, 1], F32), ('b6', [128, 1], F32),
        ('b7', [128, 1], F32), ('b8', [25, 1], F32),
    ):
        t = persist.tile(shape, dt, tag=name, name=nm(name))
        nc.sync.dma_start(t[:], ins[name])
        wt[name] = t

    PADF = 146 * 17
    pad_tiles = {}
    for l in (2, 3, 4):
        for i in range(4):
            t = persist.tile([128, PADF], BF16, tag=f"act{l}_{i}", name=nm("pad"))
            if l > 2:  # conv1 tiles arrive fully padded from the host DMA
                v = t[:].rearrange("p (h w) -> p h w", h=146, w=17)
                nc.vector.memset(v[:, 0, :], 0.0)
                nc.vector.memset(v[:, 145, :], 0.0)
                nc.vector.memset(v[:, :, 0], 0.0)
                nc.vector.memset(v[:, :, 16], 0.0)
            pad_tiles[(l, i)] = t

    act9 = persist.tile([128, nwin_total], F32, tag="act9", name="act9")
    dummy8 = persist.tile([128, 4 * 69], F32, tag="dummy8", name="dummy8")
    out_sb = persist.tile([25, nwin_total], F32, tag="out_sb", name="out_sb")

    win_order = []

    def conv_pad_layer(l, in_tiles, out_is_pad, bias, w_t, eng):
        n_t = len(in_tiles)
        n_g = max(len(t.windows) for t in in_tiles)
        outs = []
        for g in range(n_g):
            if out_is_pad:
                ot = pad_tiles[(l + 1, g)]
                outs.append(ActTile(ot[:], 32, {}))
            else:
                ot = a5p_pool.tile([128, 144 * 15], BF16, tag="a5p", name=nm("a5p"))
                outs.append(ActTile(ot[:], 32, {}))
        for (h0, hc) in CHUNKS_144:
            N = hc * 15
            pss = [psA.tile([128, 512], F32, tag="mm", name=nm("mm")) for _ in range(n_g)]
            for k in range(9):
                dh, dw = k // 3, k % 3
                for g in range(n_g):
                    for Ti, it in enumerate(in_tiles):
                        if g not in it.windows:
                            continue
                        iv = it.ap.rearrange("p (h w) -> p h w", h=146, w=17)
                        rhs = iv[32 * g:32 * g + 32, h0 + dh:h0 + dh + hc, dw:dw + 15]
                        nc.tensor.matmul(
                            pss[g][32 * Ti:32 * Ti + 32, 0:N],
                            w_t[32 * g:32 * g + 32, 32 * k:32 * k + 32],
                            rhs, start=(k == 0), stop=(k == 8), skip_group_check=True,
                            tile_position=(32 * g, 32 * Ti))
            for g in range(n_g):
                np_used = 32 * n_t
                src = pss[g][0:np_used, 0:N].rearrange("p (h w) -> p h w", h=hc, w=15)
                if out_is_pad:
                    ov = outs[g].ap.rearrange("p (h w) -> p h w", h=146, w=17)
                    dst = ov[0:np_used, 1 + h0:1 + h0 + hc, 1:16]
                else:
                    ov = outs[g].ap.rearrange("p (h w) -> p h w", h=144, w=15)
                    dst = ov[0:np_used, h0:h0 + hc, :]
                # 3:1 ACT/DVE split per chunk keeps the per-chunk evacuation
                # time (1.5us / 0.9us) under the PE's full-clock chunk fill
                # (1.63us) so the PE can stream without p-state-resetting
                # bubbles; a single engine (4 ops ~2us) cannot keep up.
                evac(dst, src, bias[0:np_used, :], 'dve' if g == 3 else 'act')
        for g in range(n_g):
            for Ti, it in enumerate(in_tiles):
                if g in it.windows:
                    outs[g].windows[Ti] = it.windows[g]
        return outs

    def pool1(a5p_tiles):
        outs = []
        for t in a5p_tiles:
            np_used = 32 * (max(t.windows) + 1)
            o = a5i_pool.tile([128, 72 * 15], BF16, tag="a5i", name=nm("a5i"))[:]
            sv = t.ap.rearrange("p (h two w) -> p h two w", h=72, two=2, w=15)
            ov = o.rearrange("p (h w) -> p h w", h=72, w=15)
            nc.vector.tensor_max(ov[0:np_used], sv[0:np_used, :, 0, :],
                                 sv[0:np_used, :, 1, :])
            outs.append(ActTile(o, 32, dict(t.windows)))
        return outs

    def l5_wave(tiles):
        pair = len(tiles) == 2
        a6_tiles = []
        n_g = len(tiles[0].windows)
        for g in range(n_g):
            o = a6_pool.tile([128, 70 * 13], BF16, tag="a6", name=nm("a6"))[:]
            a6_tiles.append(ActTile(o, 64, {}))
        for (h0, hc) in CHUNKS_70:
            N = hc * 13
            pss = [psA.tile([128, 512], F32, tag="mm", name=nm("mm")) for _ in range(n_g)]
            for k in range(9):
                dh, dw = k // 3, k % 3
                for g in range(n_g):
                    for c5, it in enumerate(tiles):
                        iv = it.ap.rearrange("p (h w) -> p h w", h=72, w=15)
                        rhs = iv[32 * g:32 * g + 32, h0 + dh:h0 + dh + hc, dw:dw + 13]
                        nc.tensor.matmul(
                            pss[g][64 * c5:64 * c5 + 64, 0:N],
                            wt['w5'][32 * g:32 * g + 32, 64 * k:64 * k + 64],
                            rhs, start=(k == 0), stop=(k == 8), skip_group_check=True,
                            tile_position=(32 * g, 64 * c5))
            for g in range(n_g):
                np_used = 128 if pair else 64
                src = pss[g][0:np_used, 0:N].rearrange("p (h w) -> p h w", h=hc, w=13)
                ov = a6_tiles[g].ap.rearrange("p (h w) -> p h w", h=70, w=13)
                if pair:
                    dst = ov[:, h0:h0 + hc, :]
                else:
                    dst = ov[0:64, h0:h0 + hc, :]
                evac(dst, src, wt['b5'][0:np_used, :], 'dve' if g == 3 else 'act')
        for g in range(n_g):
            for c5, it in enumerate(tiles):
                a6_tiles[g].windows[c5] = it.windows[g]
        if not pair:
            full = []
            for j in range(0, n_g, 2):
                t0_, t1_ = a6_tiles[j], a6_tiles[j + 1]
                o = a6_pool.tile([128, 70 * 13], BF16, tag="a6", name=nm("a6"))[:]
                nc.vector.tensor_copy(o[0:64, :], t0_.ap[0:64, :])
                nc.vector.tensor_copy(o[64:128, :], t1_.ap[0:64, :])
                full.append(ActTile(o, 64, {0: t0_.windows[0], 1: t1_.windows[0]}))
            a6_tiles = full
        return a6_tiles

    def l6_wave(tP, tQ):
        a7_tiles = [ActTile(a7_pool.tile([128, 68 * 11], BF16, tag="a7",
                                         name=nm("a7"))[:], 64, {})
                    for _ in range(2)]
        for (h0, hc) in CHUNKS_68:
            N = hc * 11
            pss = [psA.tile([128, 512], F32, tag="mm", name=nm("mm")) for _ in range(2)]
            for k in range(9):
                dh, dw = k // 3, k % 3
                for c in range(2):
                    for cc, it in enumerate((tP, tQ)):
                        iv = it.ap.rearrange("p (h w) -> p h w", h=70, w=13)
                        rhs = iv[64 * c:64 * c + 64, h0 + dh:h0 + dh + hc, dw:dw + 11]
                        nc.tensor.matmul(
                            pss[c][64 * cc:64 * cc + 64, 0:N],
                            wt['w6'][64 * c:64 * c + 64, 64 * k:64 * k + 64],
                            rhs, start=(k == 0), stop=(k == 8), skip_group_check=True,
                            tile_position=(64 * c, 64 * cc))
            for c in range(2):
                src = pss[c][:, 0:N].rearrange("p (h w) -> p h w", h=hc, w=11)
                ov = a7_tiles[c].ap.rearrange("p (h w) -> p h w", h=68, w=11)
                evac(ov[:, h0:h0 + hc, :], src, wt['b6'][:], 'act')
        for c in range(2):
            a7_tiles[c].windows = {0: tP.windows[c], 1: tQ.windows[c]}
        return a7_tiles

    REP_S = 7
    rep_state = {"rv": None, "slots": []}
    pending = deque()   # entries: (created_wave, emission_closure)
    cur_wave = [0]

    def pump(n):
        while n > 0 and pending:
            pending.popleft()[1]()
            n -= 1

    def rep_flush_emit(rv, slots):
        S = len(slots)
        ps7 = psB.tile([128, REP_S * 69], F32, tag="l7", name=nm("ps7"))[:]
        if FP8_CONV7:
            # rep storage is h-major [64, h(34), s(7), w(11)]; (oh, s) merge
            # into one stride-11 dim of count 23*7, so the DoubleRow rhs fits
            # the 3-free-dim ISA pattern: [ko(2, stride 77), oh*s, ow].
            # DoubleRow contracts rows (oh+2jj) and (oh+2jj+1) per output.
            # The matmul always computes all REP_S slots (garbage in unused
            # slots is never evacuated).
            pv = ps7.rearrange("p (h s w) -> p h s w", h=23, s=REP_S, w=3)
            w7v = wt['w7'][:].rearrange("p (i ko m) -> p i ko m", i=54, ko=2, m=128)
            for jj in range(6):
                for dw in range(9):
                    idx = jj * 9 + dw
                    sl = rv[:, 2 * jj:2 * jj + 23, 0:REP_S, dw:dw + 3]
                    rhs = type(sl)(
                        sl.tensor, sl.offset,
                        [list(sl.ap[0]), [77, 2], [11, 23 * REP_S], [1, 3]],
                        sl.const_val, sl.runtime_checks, sl.dep_tracking_offset)
                    nc.tensor.matmul(
                        pv[:, :, :, :], w7v[:, idx], rhs,
                        start=(idx == 0), stop=(idx == 53), skip_group_check=True,
                        perf_mode=mybir.MatmulPerfMode.DoubleRow,
                        tile_position=(0, 0))
            ev = [pv[:, :, s, :] for s in range(S)]
        else:
            pv = ps7.rearrange("p (s h w) -> p s h w", s=REP_S, h=23, w=3)
            for jj in range(6):
                for dw in range(9):
                    idx = jj * 9 + dw
                    nc.tensor.matmul(
                        pv[:, 0:S, :, :], wt['w7'][:, 128 * idx:128 * idx + 128],
                        rv[:, 0:S, 2 * jj:2 * jj + 23, dw:dw + 3],
                        start=(idx == 0), stop=(idx == 53), skip_group_check=True,
                        tile_position=(0, 0))
            ev = [pv[:, s, :, :] for s in range(S)]
        dv = dummy8[:].rearrange("p (s n) -> p s n", s=4, n=69)
        for s in range(S):
            col = len(win_order)
            win_order.append(slots[s])
            nc.scalar.activation(
                dv[:, s % 4, :], ev[s],
                RELU, bias=wt['b7'][:],
                scale=(1.0 / W7_SCALE) if FP8_CONV7 else 1.0,
                accum_out=act9[:, col:col + 1])

    def mk_flush(rv, slots):
        return lambda: rep_flush_emit(rv, slots)

    def l7_group(a7_pair):
        for t in a7_pair:
            tv = t.ap.rearrange("p (h two w) -> p h two w", h=34, two=2, w=11)
            for c in range(2):
                if rep_state["rv"] is None:
                    if FP8_CONV7:
                        rep = rep_pool.tile([64, 34 * REP_S * 11], FP8,
                                            tag="rep7", name=nm("rep7"))[:]
                        rep_state["rv"] = rep.rearrange(
                            "p (h s w) -> p h s w", h=34, s=REP_S, w=11)
                    else:
                        rep = rep_pool.tile([128, REP_S * 34 * 11], BF16,
                                            tag="rep7", name=nm("rep7"))[:]
                        rep_state["rv"] = rep.rearrange(
                            "p (s h w) -> p s h w", s=REP_S, h=34, w=11)
                rv = rep_state["rv"]
                s = len(rep_state["slots"])
                rep_state["slots"].append(t.windows[c])
                if FP8_CONV7:
                    nc.vector.tensor_max(
                        rv[0:64, :, s, :],
                        tv[64 * c:64 * c + 64, :, 0, :],
                        tv[64 * c:64 * c + 64, :, 1, :])
                else:
                    # pooled rows 0..33 into the low partition half and the
                    # h+1-shifted replica into the high half (conv7 reads
                    # K=128 = 2 h-rows x 64 ch per pass)
                    nc.vector.tensor_max(
                        rv[0:64, s, :, :],
                        tv[64 * c:64 * c + 64, :, 0, :],
                        tv[64 * c:64 * c + 64, :, 1, :])
                    nc.vector.tensor_max(
                        rv[64:128, s, 0:33, :],
                        tv[64 * c:64 * c + 64, 1:34, 0, :],
                        tv[64 * c:64 * c + 64, 1:34, 1, :])
                if len(rep_state["slots"]) == REP_S:
                    rv_, slots_ = rep_state["rv"], rep_state["slots"]
                    rep_state["rv"] = None
                    rep_state["slots"] = []
                    pending.append((cur_wave[0], mk_flush(rv_, slots_)))

    a6_queue = []

    def mk_l6(tP, tQ):
        def run():
            l7_group(l6_wave(tP, tQ))
        return run

    def mk_l5(tiles):
        def run():
            a6_queue.extend(l5_wave(tiles))
            while len(a6_queue) >= 2:
                tP = a6_queue.pop(0)
                tQ = a6_queue.pop(0)
                pending.append((cur_wave[0], mk_l6(tP, tQ)))
        return run

    for rep in range(repeat):
        if rep > 0:
            win_order.clear()
        a1_off = 0
        a5i_queue = []
        cur_wave[0] = 0

        def pump_lagged(n):
            # only emit deep items whose inputs are >= 1 wave old, so the PE
            # stream never queues behind a fresh cross-engine dependency
            while n > 0 and pending and pending[0][0] < cur_wave[0]:
                pending.popleft()[1]()
                n -= 1

        for b_idx in range(n_b):
            for wi, (t0, ncols) in enumerate(plan):
                np_used = 32 * ncols
                # Each issuing engine owns ONE DMA queue (~22.5 GB/s), so the
                # 2.5 MB/wave of conv1 activations is split into 8 half-tile
                # transfers spread across the 3 DMA-capable engines' queues
                # (<= 42us per queue, hidden under the ~60us wave).
                dma_eng = [nc.sync, nc.scalar, nc.gpsimd]
                k = 0
                for i in range(4):
                    dst = pad_tiles[(2, i)][:]
                    base = a1_off + i * 128 * PADF
                    lo = min(64, np_used)
                    src = ins['a1'][base: base + lo * PADF]
                    dma_eng[k % 3].dma_start(
                        dst[0:lo, :], src.rearrange("(p n) -> p n", p=lo, n=PADF))
                    k += 1
                    if np_used > 64:
                        hi = np_used - 64
                        src = ins['a1'][base + 64 * PADF: base + (64 + hi) * PADF]
                        dma_eng[k % 3].dma_start(
                            dst[64:64 + hi, :],
                            src.rearrange("(p n) -> p n", p=hi, n=PADF))
                        k += 1
                a1_off += 4 * 128 * PADF
                tiles = [ActTile(pad_tiles[(2, i)][:], 32,
                                 {s: b_idx * T + t0 + 4 * s + i
                                  for s in range(ncols)})
                         for i in range(4)]
                pump_lagged(3)
                tiles = conv_pad_layer(2, tiles, True, wt['b2'], wt['w2'], 'act')
                pump_lagged(3)
                tiles = conv_pad_layer(3, tiles, True, wt['b3'], wt['w3'], 'act')
                pump_lagged(3)
                tiles = conv_pad_layer(4, tiles, False, wt['b4'], wt['w4'], 'dve')
                a5i_queue.extend(pool1(tiles))
                while len(a5i_queue) >= 2:
                    pending.append(
                        (cur_wave[0], mk_l5([a5i_queue.pop(0), a5i_queue.pop(0)])))
                pump_lagged(4)
                cur_wave[0] += 1
            if a5i_queue:
                pending.append((cur_wave[0] - 1, mk_l5([a5i_queue.pop(0)])))
        pump(10 ** 6)
        if rep_state["slots"]:
            rv_, slots_ = rep_state["rv"], rep_state["slots"]
            rep_state["rv"] = None
            rep_state["slots"] = []
            rep_flush_emit(rv_, slots_)
        assert not a5i_queue and not a6_queue and not pending
        assert len(win_order) == nwin_total

        ps8 = psB.tile([25, nwin_total], F32, tag="l7", name=nm("ps8"))
        nc.tensor.matmul(ps8[:, :], wt['w8'][:, 0:25], act9[:, :],
                         start=True, stop=True)
        nc.scalar.activation(out_sb[:, :], ps8[:, :], IDENT, bias=wt['b8'][:])
        nc.sync.dma_start(y_ap.rearrange("t c -> c t"), out_sb[:, :])

    stack.close()
    return win_order


# --------------------------------------------------------------- entry point

_CACHE = {}


def build_program(in_map, n_b=2, T=T_PER_B, repeat=1):
    """Build + bacc-compile the SPMD program. Returns (nc, win_order)."""
    nc = bacc.Bacc("TRN2", target_bir_lowering=False, debug=False,
                   num_devices=N_CORES)
    dram = {}
    for name, arr in in_map.items():
        dram[name] = nc.dram_tensor(name, list(arr.shape),
                                    mybir.dt.from_np(arr.dtype),
                                    kind="ExternalInput")
    nwin = n_b * T
    y = nc.dram_tensor("y", [nwin, 25], mybir.dt.float32, kind="ExternalOutput")
    with tile.TileContext(nc) as tc:
        win_order = emit(tc, {k: v.ap() for k, v in dram.items()}, y.ap(),
                         n_b=n_b, T=T, repeat=repeat)
    nc.compile()
    return nc, win_order


def kernel(**inputs):
    x = np.asarray(inputs['x'])
    B, T, _ = x.shape
    in_maps, _plan = host_prepare(inputs, n_cores=N_CORES)
    key = (B, T)
    if key not in _CACHE:
        _CACHE[key] = build_program(in_maps[0], n_b=B // N_CORES, T=T)
    nc, win_order = _CACHE[key]
    res = run_bass_kernel_spmd(nc, in_maps, list(range(N_CORES)))
    order = np.asarray(win_order)
    b_per_core = B // N_CORES
    out = np.zeros((B, T, 25), np.float32)
    for c in range(N_CORES):
        yc = np.zeros((b_per_core * T, 25), np.float32)
        yc[order] = res.results[c]['y']
        out[c * b_per_core:(c + 1) * b_per_core] = yc.reshape(b_per_core, T, 25)
    return out

